# revision 15
# baseline (speedup 1.0000x reference)
"""Trainium2 Bass kernel for nn_AdversarialLoss (pairwise JS loss over softmaxes).

Strategy (8 NeuronCores, no collectives):
  - Only pairs (i<j) with equal labels contribute. Pairs exist only inside label
    groups, so groups are assigned to cores (split if needed) and each core
    computes a partial sum over its own pairs using only its own rows of x.
  - Per core the device computes, for its (padded) row set:
        y   = x_rows @ W.T + b                      (bf16 matmul, f32 accum)
        G   = y @ seen_att.T                        (bf16 matmul)
        u   = 5 * G / (|y_i| * |sa_c|)              (= logits/TEMP)
        e   = exp(u - max), se = sum(e)             (softmax numerator)
        negh_i = sum_c p*logP = (sum e*u)/se - lse
        q_n = p_i + p_j  via fp32 matmul S'.T @ e   (S' = pair-select * 1/se)
        v_n = sum_c q*ln(q)
    and returns v [L] and negh [R]; the host combines
        loss = 16/cnt * ( sum_pairs(0.5*(negh_i+negh_j)) + cnt*ln2 - 0.5*sum v )
  - W.T / seen_att.T are read by every core (redundant, bf16 to halve bytes);
    x / pair-selection are sharded. Host does only O(B^2) index bookkeeping,
    layout swizzles, and the final dot with 0/1 pair-count weights.

Self-contained: hardcodes shapes from the problem spec (x[256,2048],
W[512,2048], b[512], seen_att[1024,512], labels[256]).
"""

import numpy as np
import ml_dtypes
from contextlib import ExitStack

import concourse.bacc as bacc
import concourse.tile as tile
import concourse.mybir as mybir
from concourse import masks
from concourse.bass_utils import run_bass_kernel_spmd
from concourse.hw_specs import get_activation_tables as _real_act_tables


def _act_tables_ln_exp_only(module_arch):
    """Keep only the one act-func set that covers ln+exp+square+copy so the
    table-load pass emits a single LoadActFuncSet instead of ping-ponging
    between per-function sets. Positions are preserved so set ids stay valid."""
    tables = _real_act_tables(module_arch)
    out = {}
    for name, funcs in tables.items():
        if name == "natural_log_exp_and_others":
            out[name] = funcs
        else:
            out[name] = set()
    return out


# NOTE: forcing every activation into act-func-set 6 ("natural_log_exp_and_
# others") costs ~10x accuracy on HW (rel err 2e-3 vs 2e-4) - its ln/exp
# tables are lower-precision than the per-function sets. Left disabled.

dt = mybir.dt
AF = mybir.ActivationFunctionType
ALU = mybir.AluOpType
AX = mybir.AxisListType

B, D, ATT, C = 256, 2048, 512, 1024
KD, KA = D // 128, ATT // 128   # K-chunks for the two matmuls
R = 128                         # padded rows per core (fixed -> one cached NEFF)
QCHUNK = 128                    # pairs per Q tile
N_CORES = 8

_BF = ml_dtypes.bfloat16

_prog_cache: dict = {}


def _build_program(NQ: int):
    """Build the (input-independent) 8-core SPMD Bass program for NQ pair-tiles."""
    if NQ in _prog_cache:
        return _prog_cache[NQ]
    L = NQ * QCHUNK
    nc = bacc.Bacc("TRN2", target_bir_lowering=False, debug=False,
                   num_devices=N_CORES)

    xt_d = nc.dram_tensor("xt", [128, KD * R], dt.bfloat16, kind="ExternalInput")
    wt_d = nc.dram_tensor("wt", [128, KD * ATT], dt.bfloat16, kind="ExternalInput")
    sat_d = nc.dram_tensor("sat", [128, KA * C], dt.bfloat16, kind="ExternalInput")
    b_d = nc.dram_tensor("bias", [1, ATT], dt.bfloat16, kind="ExternalInput")
    st_d = nc.dram_tensor("st", [R, L], dt.float32, kind="ExternalInput")
    outv_d = nc.dram_tensor("outv", [L, 1], dt.float32, kind="ExternalOutput")
    outh_d = nc.dram_tensor("outh", [R, 1], dt.float32, kind="ExternalOutput")

    with tile.TileContext(nc) as tc, ExitStack() as ctx:
        io = ctx.enter_context(tc.tile_pool(name="io", bufs=1))
        wk = ctx.enter_context(tc.tile_pool(name="wk", bufs=1))
        ps = ctx.enter_context(tc.tile_pool(name="ps", bufs=1, space="PSUM"))

        # ---- input DMAs (HWDGE), ordered for earliest dependency release:
        # xt (M1 lhsT), sat halves (sa-norm chain), wt in 8 chunks pipelined
        # with M1, small tensors in between. ----
        sat_sb = io.tile([128, KA * C], dt.bfloat16)
        nc.sync.dma_start(sat_sb[:, :2 * C], sat_d.ap()[:, :2 * C])
        nc.sync.dma_start(sat_sb[:, 2 * C:], sat_d.ap()[:, 2 * C:])
        xt_sb = io.tile([128, KD * R], dt.bfloat16)
        nc.sync.dma_start(xt_sb[:], xt_d.ap())
        st_sb = io.tile([R, L], dt.float32)
        nc.sync.dma_start(st_sb[:], st_d.ap())
        b_sb = io.tile([1, ATT], dt.bfloat16)
        nc.sync.dma_start(b_sb[:], b_d.ap())
        wt_sb = io.tile([128, KD * ATT], dt.bfloat16)
        wsl = [slice(w * 2 * ATT, (w + 1) * 2 * ATT) for w in range(8)]
        for w in range(8):
            nc.sync.dma_start(wt_sb[:, wsl[w]], wt_d.ap()[:, wsl[w]])

        # ---- constants ----
        ident = wk.tile([128, 128], dt.bfloat16)
        masks.make_identity(nc, ident[:])
        ones1R_bf = wk.tile([1, R], dt.bfloat16)
        nc.gpsimd.memset(ones1R_bf[:], 1.0)
        ones1R_r = wk.tile([1, R], dt.float32r)
        nc.vector.tensor_copy(ones1R_r[:], ones1R_bf[:])
        ones128_f = wk.tile([128, 1], dt.float32)
        nc.gpsimd.memset(ones128_f[:], 1.0)
        ones128_r = wk.tile([128, 1], dt.float32r)
        nc.vector.tensor_copy(ones128_r[:], ones128_f[:])
        st_r = wk.tile([R, L], dt.float32r)
        nc.vector.tensor_copy(st_r[:], st_sb[:])  # 0/1/2 values: exact in f32r

        # ---- seen_att column norms: nsq_c = sum_a sa[c,a]^2 (f32r matmuls) ----
        sasq = [wk.tile([128, C], dt.float32r, name=f"sasq{j}") for j in range(KA)]
        for j in range(KA):  # DVE keeps Square off ACT (fewer table loads)
            src = sat_sb[:, j * C:(j + 1) * C]
            nc.vector.tensor_tensor(sasq[j][:], src, src, ALU.mult)
        nsq_ps = ps.tile([1, C], dt.float32, tag="big", bufs=2)
        for j in range(KA):
            for h in range(2):
                nc.tensor.matmul(nsq_ps[:, h * 512:(h + 1) * 512],
                                 ones128_r[:], sasq[j][:, h * 512:(h + 1) * 512],
                                 start=(j == 0), stop=(j == KA - 1))
        # rn'_c = 1/max(|sa_c|, 1e-12) = exp(-0.5*ln(max(nsq, 1e-24)))
        nsqc = wk.tile([1, C], dt.float32)
        nc.vector.tensor_scalar_max(nsqc[:], nsq_ps[:], 1e-24)
        lnn = wk.tile([1, C], dt.float32)
        nc.scalar.activation(lnn[:], nsqc[:], AF.Ln)
        # ---- M1: y = x @ W.T + b ----
        y_ps = ps.tile([R, ATT], dt.float32, tag="y")
        for k in range(KD):
            nc.tensor.matmul(y_ps[:], xt_sb[:, k * R:(k + 1) * R],
                             wt_sb[:, k * ATT:(k + 1) * ATT],
                             start=(k == 0), stop=False)
        nc.tensor.matmul(y_ps[:], ones1R_bf[:], b_sb[:], start=False, stop=True)

        # y -> bf16, transpose to yT for M2
        y_bf = wk.tile([R, ATT], dt.bfloat16)
        nc.vector.tensor_copy(y_bf[:], y_ps[:])
        yt_sb = wk.tile([128, KA * R], dt.bfloat16)
        for j in range(KA):
            yt_ps = ps.tile([128, R], dt.bfloat16, tag="t", bufs=2, name=f"ytp{j}")
            nc.tensor.transpose(yt_ps[:], y_bf[:, j * 128:(j + 1) * 128], ident[:])
            nc.vector.tensor_copy(yt_sb[:, j * R:(j + 1) * R], yt_ps[:])

        # row norms from bf16 y (stt-accum; keeps Square off ACT):
        # rn5_i = 5/max(|y_i|, 1e-12) = exp(-0.5*ln(max(ssq,1e-24)) + ln5)
        scr_y = wk.tile([R, ATT], dt.float32)
        rowssq = wk.tile([R, 1], dt.float32)
        nc.vector.scalar_tensor_tensor(scr_y[:], y_bf[:], 1.0, y_bf[:],
                                       op0=ALU.mult, op1=ALU.mult,
                                       accum_out=rowssq[:])
        rsqc = wk.tile([R, 1], dt.float32)
        nc.vector.tensor_scalar_max(rsqc[:], rowssq[:], 1e-24)
        lnr = wk.tile([R, 1], dt.float32)
        nc.scalar.activation(lnr[:], rsqc[:], AF.Ln)
        # Exp ops grouped after the two Ln ops: 3 act-table loads total
        rnp = wk.tile([1, C], dt.float32r)
        nc.scalar.activation(rnp[:], lnn[:], AF.Exp, scale=-0.5)
        ln5 = wk.tile([R, 1], dt.float32)
        nc.gpsimd.memset(ln5[:], float(np.log(5.0)))
        rn5 = wk.tile([R, 1], dt.float32)
        nc.scalar.activation(rn5[:], lnr[:], AF.Exp, scale=-0.5, bias=ln5[:])

        # ---- M2: G = y @ saT ----
        g_ps = ps.tile([R, C], dt.float32, tag="big", bufs=2)
        for j in range(KA):
            for h in range(2):
                nc.tensor.matmul(g_ps[:, h * 512:(h + 1) * 512],
                                 yt_sb[:, j * R:(j + 1) * R],
                                 sat_sb[:, j * C + h * 512: j * C + (h + 1) * 512],
                                 start=(j == 0), stop=(j == KA - 1))

        # broadcast to all partitions: RN = ones[R,1] @ rnp  (f32r matmul)
        rn_ps = ps.tile([R, C], dt.float32, tag="big", bufs=2)
        for h in range(2):
            nc.tensor.matmul(rn_ps[:, h * 512:(h + 1) * 512],
                             ones1R_r[:], rnp[:, h * 512:(h + 1) * 512],
                             start=True, stop=True)
        RN = wk.tile([R, C], dt.float32)
        nc.vector.tensor_copy(RN[:], rn_ps[:])

        # ---- u = (G * rn5_i) * rn'_c ; softmax (u in [-5,5]: no max needed) ----
        u = wk.tile([R, C], dt.float32)
        nc.vector.scalar_tensor_tensor(u[:], g_ps[:], rn5[:], RN[:],
                                       op0=ALU.mult, op1=ALU.mult)
        e = wk.tile([R, C], dt.float32)
        se = wk.tile([R, 1], dt.float32)
        nc.scalar.activation(e[:], u[:], AF.Exp, accum_out=se[:])
        rse = wk.tile([R, 1], dt.float32)
        nc.vector.reciprocal(rse[:], se[:])
        p_r = wk.tile([R, C], dt.float32r)
        p_r_inst = nc.vector.tensor_scalar_mul(p_r[:], e[:], rse[:])

        # ---- pairs: q = S.T @ P (f32r), v = sum_c q*ln(q) ----
        for qi in range(NQ):
            q_ps = ps.tile([QCHUNK, C], dt.float32, tag="big", bufs=2,
                           name=f"qps{qi}")
            v = wk.tile([QCHUNK, 1], dt.float32, tag="v", bufs=2, name=f"v{qi}")
            vh = [wk.tile([QCHUNK, 1], dt.float32, tag=f"vh{h}", bufs=2,
                          name=f"vh{qi}_{h}") for h in range(2)]
            for h in range(2):
                nc.tensor.matmul(q_ps[:, h * 512:(h + 1) * 512],
                                 st_r[:, qi * QCHUNK:(qi + 1) * QCHUNK],
                                 p_r[:, h * 512:(h + 1) * 512],
                                 start=True, stop=True)
            for h in range(2):
                lnq = wk.tile([QCHUNK, 512], dt.float32, tag="lnq", bufs=2,
                              name=f"lnq{qi}_{h}")
                scr3 = wk.tile([QCHUNK, 512], dt.float32, tag="scr3", bufs=2,
                               name=f"scr3{qi}_{h}")
                nc.scalar.activation(lnq[:], q_ps[:, h * 512:(h + 1) * 512], AF.Ln)
                nc.vector.scalar_tensor_tensor(
                    scr3[:], q_ps[:, h * 512:(h + 1) * 512], 1.0, lnq[:],
                    op0=ALU.mult, op1=ALU.mult, accum_out=vh[h][:])
            nc.vector.tensor_tensor(v[:], vh[0][:], vh[1][:], ALU.add)
            nc.sync.dma_start(outv_d.ap()[qi * QCHUNK:(qi + 1) * QCHUNK, :], v[:])

        # ---- negh = (sum_c e*u)/se - ln(se)  (emitted last: fills gaps) ----
        scr2 = wk.tile([R, C], dt.float32)
        t1 = wk.tile([R, 1], dt.float32)
        t1_inst = nc.vector.scalar_tensor_tensor(scr2[:], e[:], 1.0, u[:],
                                                 op0=ALU.mult, op1=ALU.mult,
                                                 accum_out=t1[:])
        from concourse.tile_rust import add_dep_helper
        add_dep_helper(t1_inst.ins, p_r_inst.ins,
                       reason="keep negh accumulation off the pair critical path")
        lnse = wk.tile([R, 1], dt.float32)
        nc.scalar.activation(lnse[:], se[:], AF.Ln)
        negh = wk.tile([R, 1], dt.float32)
        nc.vector.scalar_tensor_tensor(negh[:], t1[:], rse[:], lnse[:],
                                       op0=ALU.mult, op1=ALU.subtract)
        nc.sync.dma_start(outh_d.ap(), negh[:])

    nc.compile()
    _prog_cache[NQ] = nc
    return nc


def _shard_pairs(labels):
    groups: dict = {}
    for i, g in enumerate(labels.tolist()):
        groups.setdefault(g, []).append(i)
    group_pairs = []
    for rows in groups.values():
        ps = [(rows[a], rows[b])
              for a in range(len(rows)) for b in range(a + 1, len(rows))]
        if ps:
            group_pairs.append(ps)
    cnt = sum(len(p) for p in group_pairs)
    if cnt == 0:
        return None, 0
    group_pairs.sort(key=len, reverse=True)
    core_pairs = [[] for _ in range(N_CORES)]
    cap = max(1, (cnt + N_CORES - 1) // N_CORES)
    for ps in group_pairs:
        k = min(range(N_CORES), key=lambda kk: len(core_pairs[kk]))
        while len(ps) > cap:
            core_pairs[k].extend(ps[:cap])
            ps = ps[cap:]
            k = min(range(N_CORES), key=lambda kk: len(core_pairs[kk]))
        core_pairs[k].extend(ps)
    return core_pairs, cnt


def _swizzle_kmaj(a2d, kchunks):
    """[Ktot, N] -> [128, kchunks*N] with element (p, k*N+n) = a[k*128+p, n]."""
    ktot, n = a2d.shape
    assert ktot == kchunks * 128
    return np.ascontiguousarray(
        a2d.reshape(kchunks, 128, n).transpose(1, 0, 2).reshape(128, kchunks * n))


def prep_inputs(x, labels, W, b, seen_att):
    """Host-side sharding/layout. Returns (in_maps, per_core_meta, cnt, NQ)."""
    core_pairs, cnt = _shard_pairs(labels)
    if cnt == 0:
        return None, None, 0, 0
    NQ = (max(len(p) for p in core_pairs) + QCHUNK - 1) // QCHUNK
    L = NQ * QCHUNK
    wt = _swizzle_kmaj(np.ascontiguousarray(W.T), KD).astype(_BF)
    sat = _swizzle_kmaj(np.ascontiguousarray(seen_att.T), KA).astype(_BF)
    b_row = np.asarray(b, np.float32).reshape(1, ATT).astype(_BF)
    in_maps, metas = [], []
    for k in range(N_CORES):
        pairs = core_pairs[k]
        rows = sorted({r for p in pairs for r in p})
        assert len(rows) <= R, f"core {k}: row set {len(rows)} exceeds {R}"
        ridx = {r: a for a, r in enumerate(rows)}
        xk = np.zeros((D, R), np.float32)
        if rows:
            xk[:, :len(rows)] = np.asarray(x, np.float32)[rows].T
        st = np.zeros((R, L), np.float32)
        for n, (i, j) in enumerate(pairs):
            st[ridx[i], n] = 1.0
            st[ridx[j], n] = 1.0
        for n in range(len(pairs), L):
            st[0, n] = 2.0  # benign padding: q = 2*p_row0 > 0
        wrow = np.zeros(R, np.float32)
        for (i, j) in pairs:
            wrow[ridx[i]] += 1.0
            wrow[ridx[j]] += 1.0
        in_maps.append({
            "xt": _swizzle_kmaj(xk, KD).astype(_BF),
            "wt": wt, "sat": sat, "bias": b_row,
            "st": st,
        })
        metas.append((len(pairs), wrow))
    return in_maps, metas, cnt, NQ


def aggregate(results, metas, cnt):
    total = 0.0
    for res, (npair, wrow) in zip(results, metas):
        v = np.asarray(res["outv"], np.float64).reshape(-1)
        negh = np.asarray(res["outh"], np.float64).reshape(-1)
        total += 0.5 * float(wrow.astype(np.float64) @ negh)
        total -= 0.5 * float(v[:npair].sum())
    total += cnt * np.log(2.0)
    return np.float32(total / cnt * 16.0)


def kernel(x, gt_s_labels, W, b, seen_att):
    labels = np.asarray(gt_s_labels)
    in_maps, metas, cnt, NQ = prep_inputs(x, labels, W, b, seen_att)
    if cnt == 0:
        return np.float32(0.0)
    nc = _build_program(NQ)
    res = run_bass_kernel_spmd(nc, in_maps, core_ids=list(range(N_CORES)))
    return aggregate(res.results, metas, cnt)


if __name__ == "__main__":
    data = np.load("/root/problem/inputs_cache.npz")
    out = kernel(data["x"], data["gt_s_labels"], data["W"], data["b"],
                 data["seen_att"])
    print("kernel loss:", out)


# revision 22
# speedup vs baseline: 1.1974x; 1.1974x over previous
"""Trainium2 Bass kernel for nn_AdversarialLoss (pairwise JS loss over softmaxes).

Strategy (8 NeuronCores, no collectives):
  - Only pairs (i<j) with equal labels contribute. Pairs exist only inside label
    groups, so groups are assigned to cores (split if needed) and each core
    computes a partial sum over its own pairs using only its own rows of x.
  - Per core the device computes, for its (padded) row set:
        y   = x_rows @ W.T + b                      (bf16 matmul, f32 accum)
        G   = y @ seen_att.T                        (bf16 matmul)
        u   = 5 * G / (|y_i| * |sa_c|)              (= logits/TEMP)
        e   = exp(u - max), se = sum(e)             (softmax numerator)
        negh_i = sum_c p*logP = (sum e*u)/se - lse
        q_n = p_i + p_j  via fp32 matmul S'.T @ e   (S' = pair-select * 1/se)
        v_n = sum_c q*ln(q)
    and returns v [L] and negh [R]; the host combines
        loss = 16/cnt * ( sum_pairs(0.5*(negh_i+negh_j)) + cnt*ln2 - 0.5*sum v )
  - W.T / seen_att.T are read by every core (redundant, bf16 to halve bytes);
    x / pair-selection are sharded. Host does only O(B^2) index bookkeeping,
    layout swizzles, and the final dot with 0/1 pair-count weights.

Self-contained: hardcodes shapes from the problem spec (x[256,2048],
W[512,2048], b[512], seen_att[1024,512], labels[256]).
"""

import numpy as np
import ml_dtypes
from contextlib import ExitStack

import concourse.bacc as bacc
import concourse.tile as tile
import concourse.mybir as mybir
from concourse import masks
from concourse.bass_utils import run_bass_kernel_spmd
from concourse.hw_specs import get_activation_tables as _real_act_tables


def _act_tables_ln_exp_only(module_arch):
    """Keep only the one act-func set that covers ln+exp+square+copy so the
    table-load pass emits a single LoadActFuncSet instead of ping-ponging
    between per-function sets. Positions are preserved so set ids stay valid."""
    tables = _real_act_tables(module_arch)
    out = {}
    for name, funcs in tables.items():
        if name == "natural_log_exp_and_others":
            out[name] = funcs
        else:
            out[name] = set()
    return out


# NOTE: forcing every activation into act-func-set 6 ("natural_log_exp_and_
# others") costs ~10x accuracy on HW (rel err 2e-3 vs 2e-4) - its ln/exp
# tables are lower-precision than the per-function sets. Left disabled.

dt = mybir.dt
AF = mybir.ActivationFunctionType
ALU = mybir.AluOpType
AX = mybir.AxisListType

B, D, ATT, C = 256, 2048, 512, 1024
KD, KA = D // 128, ATT // 128   # K-chunks for the two matmuls
R = 128                         # padded rows per core (fixed -> one cached NEFF)
QCHUNK = 128                    # pairs per Q tile
N_CORES = 8

_BF = ml_dtypes.bfloat16
_F8 = ml_dtypes.float8_e4m3
M1_SCALE = 16.0  # pre-scale W/b so fp8 sees normal-range values; l2norm cancels it

_prog_cache: dict = {}


def _build_program(NQ: int):
    """Build the (input-independent) 8-core SPMD Bass program for NQ pair-tiles."""
    if NQ in _prog_cache:
        return _prog_cache[NQ]
    L = NQ * QCHUNK
    nc = bacc.Bacc("TRN2", target_bir_lowering=False, debug=False,
                   num_devices=N_CORES)

    xt_d = nc.dram_tensor("xt", [128, KD * R], dt.float8e4, kind="ExternalInput")
    wt_d = nc.dram_tensor("wt", [128, KD * ATT], dt.float8e4, kind="ExternalInput")
    sat_d = nc.dram_tensor("sat", [128, KA * C], dt.bfloat16, kind="ExternalInput")
    b_d = nc.dram_tensor("bias", [1, ATT], dt.float8e4, kind="ExternalInput")
    st_d = nc.dram_tensor("st", [R, L], dt.float32, kind="ExternalInput")
    outv_d = nc.dram_tensor("outv", [L, 1], dt.float32, kind="ExternalOutput")
    outh_d = nc.dram_tensor("outh", [R, 1], dt.float32, kind="ExternalOutput")

    with tile.TileContext(nc) as tc, ExitStack() as ctx:
        io = ctx.enter_context(tc.tile_pool(name="io", bufs=1))
        wk = ctx.enter_context(tc.tile_pool(name="wk", bufs=1))
        ps = ctx.enter_context(tc.tile_pool(name="ps", bufs=1, space="PSUM"))

        # ---- input DMAs (HWDGE), ordered for earliest dependency release:
        # xt (M1 lhsT), sat halves (sa-norm chain), wt in 8 chunks pipelined
        # with M1, small tensors in between. ----
        xt_sb = io.tile([128, KD * R], dt.float8e4)
        nc.sync.dma_start(xt_sb[:], xt_d.ap())
        sat_sb = io.tile([128, KA * C], dt.bfloat16)
        nc.sync.dma_start(sat_sb[:, :2 * C], sat_d.ap()[:, :2 * C])
        nc.sync.dma_start(sat_sb[:, 2 * C:], sat_d.ap()[:, 2 * C:])
        st_sb = io.tile([R, L], dt.float32)
        nc.sync.dma_start(st_sb[:], st_d.ap())
        b_sb = io.tile([1, ATT], dt.float8e4)
        nc.sync.dma_start(b_sb[:], b_d.ap())
        wt_sb = io.tile([128, KD * ATT], dt.float8e4)
        wsl = [slice(w * 2 * ATT, (w + 1) * 2 * ATT) for w in range(8)]
        for w in range(8):
            nc.sync.dma_start(wt_sb[:, wsl[w]], wt_d.ap()[:, wsl[w]])

        # ---- constants ----
        ident = wk.tile([128, 128], dt.bfloat16)
        masks.make_identity(nc, ident[:])
        ones1R_f8 = wk.tile([1, R], dt.float8e4)
        nc.gpsimd.memset(ones1R_f8[:], 1.0)
        ones128_f = wk.tile([128, 1], dt.float32)
        nc.gpsimd.memset(ones128_f[:], 1.0)
        ones128_r = wk.tile([128, 1], dt.float32r)
        nc.vector.tensor_copy(ones128_r[:], ones128_f[:])
        st_r = wk.tile([R, L], dt.float32r)
        nc.vector.tensor_copy(st_r[:], st_sb[:])  # 0/1/2 values: exact in f32r

        # ---- seen_att column norms: nsq_c = sum_a sa[c,a]^2 (f32r matmuls) ----
        sasq = [wk.tile([128, C], dt.float32r, name=f"sasq{j}") for j in range(KA)]
        for j in range(KA):  # DVE keeps Square off ACT (fewer table loads)
            src = sat_sb[:, j * C:(j + 1) * C]
            nc.vector.tensor_tensor(sasq[j][:], src, src, ALU.mult)
        nsq_ps = ps.tile([1, C], dt.float32, tag="big", bufs=2)
        for j in range(KA):
            for h in range(2):
                nc.tensor.matmul(nsq_ps[:, h * 512:(h + 1) * 512],
                                 ones128_r[:], sasq[j][:, h * 512:(h + 1) * 512],
                                 start=(j == 0), stop=(j == KA - 1))
        # rn'_c = 1/max(|sa_c|, 1e-12) = exp(-0.5*ln(nsq + 1e-24))
        eps1 = wk.tile([1, 1], dt.float32)
        nc.gpsimd.memset(eps1[:], 1e-24)
        lnn = wk.tile([1, C], dt.float32)
        nc.scalar.activation(lnn[:], nsq_ps[:], AF.Ln, bias=eps1[:])
        # ---- M1: y = x @ W.T + b ----
        y_ps = ps.tile([R, ATT], dt.float32, tag="y")
        for k in range(KD):
            nc.tensor.matmul(y_ps[:], xt_sb[:, k * R:(k + 1) * R],
                             wt_sb[:, k * ATT:(k + 1) * ATT],
                             start=(k == 0), stop=False)
        nc.tensor.matmul(y_ps[:], ones1R_f8[:], b_sb[:], start=False, stop=True)

        # y -> bf16, transpose to yT for M2
        y_bf = wk.tile([R, ATT], dt.bfloat16)
        nc.vector.tensor_copy(y_bf[:], y_ps[:])
        yt_sb = wk.tile([128, KA * R], dt.bfloat16)
        for j in range(KA):
            yt_ps = ps.tile([128, R], dt.bfloat16, tag="t", bufs=1, name=f"ytp{j}")
            nc.tensor.transpose(yt_ps[:], y_bf[:, j * 128:(j + 1) * 128], ident[:])
            nc.vector.tensor_copy(yt_sb[:, j * R:(j + 1) * R], yt_ps[:])

        # row norms from bf16 y (stt-accum; keeps Square off ACT):
        # rn5_i = 5/max(|y_i|, 1e-12) = exp(-0.5*ln(max(ssq,1e-24)) + ln5)
        scr_y = wk.tile([R, ATT], dt.float32)
        rowssq = wk.tile([R, 1], dt.float32)
        nc.vector.scalar_tensor_tensor(scr_y[:], y_bf[:], 1.0, y_bf[:],
                                       op0=ALU.mult, op1=ALU.mult,
                                       accum_out=rowssq[:])
        from concourse.tile_rust import add_dep_helper as _adh
        epsR = wk.tile([R, 1], dt.float32)
        nc.gpsimd.memset(epsR[:], 1e-24)
        lnr = wk.tile([R, 1], dt.float32)
        nc.scalar.activation(lnr[:], rowssq[:], AF.Ln, bias=epsR[:])
        # Exp ops grouped after the two Ln ops: 3 act-table loads total
        rnp = wk.tile([1, C], dt.float32r)
        nc.scalar.activation(rnp[:], lnn[:], AF.Exp, scale=-0.5)
        ln5 = wk.tile([R, 1], dt.float32)
        nc.gpsimd.memset(ln5[:], float(np.log(5.0)))
        rn5 = wk.tile([R, 1], dt.float32)
        nc.scalar.activation(rn5[:], lnr[:], AF.Exp, scale=-0.5, bias=ln5[:])

        # ---- M2: G = y @ saT ----
        g_ps = ps.tile([R, C], dt.float32, tag="big", bufs=2)
        for j in range(KA):
            for h in range(2):
                nc.tensor.matmul(g_ps[:, h * 512:(h + 1) * 512],
                                 yt_sb[:, j * R:(j + 1) * R],
                                 sat_sb[:, j * C + h * 512: j * C + (h + 1) * 512],
                                 start=(j == 0), stop=(j == KA - 1))

        # broadcast rn' to all partitions on the (otherwise idle) GpSimd engine
        RN = wk.tile([R, C], dt.float32r)
        nc.gpsimd.partition_broadcast(RN[:], rnp[:])

        # ---- u = (G * rn5_i) * rn'_c ; softmax (u in [-5,5]: no max needed).
        # Everything is split into C-halves so ACT/DVE/PE pipeline. ----
        u = wk.tile([R, C], dt.float32)
        seh = [wk.tile([R, 1], dt.float32, name=f"seh{h}") for h in range(2)]
        e = wk.tile([R, C], dt.float32)
        for h in range(2):
            sl = slice(h * 512, (h + 1) * 512)
            nc.vector.scalar_tensor_tensor(u[:, sl], g_ps[:, sl], rn5[:], RN[:, sl],
                                           op0=ALU.mult, op1=ALU.mult)
            nc.scalar.activation(e[:, sl], u[:, sl], AF.Exp, accum_out=seh[h][:])
        se = wk.tile([R, 1], dt.float32)
        nc.vector.tensor_tensor(se[:], seh[0][:], seh[1][:], ALU.add)
        rse = wk.tile([R, 1], dt.float32)
        nc.vector.reciprocal(rse[:], se[:])
        p_r = wk.tile([R, C], dt.float32r)
        p_r_inst = None
        for h in range(2):
            sl = slice(h * 512, (h + 1) * 512)
            p_r_inst = nc.vector.tensor_scalar_mul(p_r[:, sl], e[:, sl], rse[:])

        # ---- pairs: q = S.T @ P (f32r), v = sum_c q*ln(q) ----
        for qi in range(NQ):
            v = wk.tile([QCHUNK, 1], dt.float32, tag="v", bufs=2, name=f"v{qi}")
            vh = [wk.tile([QCHUNK, 1], dt.float32, tag=f"vh{h}", bufs=2,
                          name=f"vh{qi}_{h}") for h in range(2)]
            for h in range(2):
                q_ps = ps.tile([QCHUNK, 512], dt.float32, tag=f"qh{h}", bufs=1,
                               name=f"qps{qi}_{h}")
                nc.tensor.matmul(q_ps[:],
                                 st_r[:, qi * QCHUNK:(qi + 1) * QCHUNK],
                                 p_r[:, h * 512:(h + 1) * 512],
                                 start=True, stop=True)
                lnq = wk.tile([QCHUNK, 512], dt.float32, tag="lnq", bufs=2,
                              name=f"lnq{qi}_{h}")
                scr3 = wk.tile([QCHUNK, 512], dt.float32, tag="scr3", bufs=2,
                               name=f"scr3{qi}_{h}")
                nc.scalar.activation(lnq[:], q_ps[:], AF.Ln)
                nc.vector.scalar_tensor_tensor(
                    scr3[:], q_ps[:], 1.0, lnq[:],
                    op0=ALU.mult, op1=ALU.mult, accum_out=vh[h][:])
            nc.vector.tensor_tensor(v[:], vh[0][:], vh[1][:], ALU.add)
            nc.sync.dma_start(outv_d.ap()[qi * QCHUNK:(qi + 1) * QCHUNK, :], v[:])

        # ---- negh = (sum_c e*u)/se - ln(se)  (emitted last: fills gaps) ----
        scr2 = wk.tile([R, C], dt.float32)
        t1h = [wk.tile([R, 1], dt.float32, name=f"t1h{h}") for h in range(2)]
        for h in range(2):
            sl = slice(h * 512, (h + 1) * 512)
            t1_inst = nc.vector.scalar_tensor_tensor(scr2[:, sl], e[:, sl], 1.0,
                                                     u[:, sl], op0=ALU.mult,
                                                     op1=ALU.mult,
                                                     accum_out=t1h[h][:])
            _adh(t1_inst.ins, p_r_inst.ins,
                 reason="keep negh accumulation off the pair critical path")
        t1 = wk.tile([R, 1], dt.float32)
        nc.vector.tensor_tensor(t1[:], t1h[0][:], t1h[1][:], ALU.add)
        lnse = wk.tile([R, 1], dt.float32)
        nc.scalar.activation(lnse[:], se[:], AF.Ln)
        negh = wk.tile([R, 1], dt.float32)
        nc.vector.scalar_tensor_tensor(negh[:], t1[:], rse[:], lnse[:],
                                       op0=ALU.mult, op1=ALU.subtract)
        nc.sync.dma_start(outh_d.ap(), negh[:])

    nc.compile()
    _prog_cache[NQ] = nc
    return nc


def _shard_pairs(labels):
    groups: dict = {}
    for i, g in enumerate(labels.tolist()):
        groups.setdefault(g, []).append(i)
    group_pairs = []
    for rows in groups.values():
        ps = [(rows[a], rows[b])
              for a in range(len(rows)) for b in range(a + 1, len(rows))]
        if ps:
            group_pairs.append(ps)
    cnt = sum(len(p) for p in group_pairs)
    if cnt == 0:
        return None, 0
    group_pairs.sort(key=len, reverse=True)
    core_pairs = [[] for _ in range(N_CORES)]
    cap = max(1, (cnt + N_CORES - 1) // N_CORES)
    for ps in group_pairs:
        k = min(range(N_CORES), key=lambda kk: len(core_pairs[kk]))
        while len(ps) > cap:
            core_pairs[k].extend(ps[:cap])
            ps = ps[cap:]
            k = min(range(N_CORES), key=lambda kk: len(core_pairs[kk]))
        core_pairs[k].extend(ps)
    return core_pairs, cnt


def _swizzle_kmaj(a2d, kchunks):
    """[Ktot, N] -> [128, kchunks*N] with element (p, k*N+n) = a[k*128+p, n]."""
    ktot, n = a2d.shape
    assert ktot == kchunks * 128
    return np.ascontiguousarray(
        a2d.reshape(kchunks, 128, n).transpose(1, 0, 2).reshape(128, kchunks * n))


def prep_inputs(x, labels, W, b, seen_att):
    """Host-side sharding/layout. Returns (in_maps, per_core_meta, cnt, NQ)."""
    core_pairs, cnt = _shard_pairs(labels)
    if cnt == 0:
        return None, None, 0, 0
    NQ = (max(len(p) for p in core_pairs) + QCHUNK - 1) // QCHUNK
    L = NQ * QCHUNK
    wt = (_swizzle_kmaj(np.ascontiguousarray(W.T), KD) * M1_SCALE).astype(_F8)
    sat = _swizzle_kmaj(np.ascontiguousarray(seen_att.T), KA).astype(_BF)
    b_row = (np.asarray(b, np.float32).reshape(1, ATT) * M1_SCALE).astype(_F8)
    in_maps, metas = [], []
    for k in range(N_CORES):
        pairs = core_pairs[k]
        rows = sorted({r for p in pairs for r in p})
        assert len(rows) <= R, f"core {k}: row set {len(rows)} exceeds {R}"
        ridx = {r: a for a, r in enumerate(rows)}
        xk = np.zeros((D, R), np.float32)
        if rows:
            xk[:, :len(rows)] = np.asarray(x, np.float32)[rows].T
        st = np.zeros((R, L), np.float32)
        for n, (i, j) in enumerate(pairs):
            st[ridx[i], n] = 1.0
            st[ridx[j], n] = 1.0
        for n in range(len(pairs), L):
            st[0, n] = 2.0  # benign padding: q = 2*p_row0 > 0
        wrow = np.zeros(R, np.float32)
        for (i, j) in pairs:
            wrow[ridx[i]] += 1.0
            wrow[ridx[j]] += 1.0
        in_maps.append({
            "xt": _swizzle_kmaj(xk, KD).astype(_F8),
            "wt": wt, "sat": sat, "bias": b_row,
            "st": st,
        })
        metas.append((len(pairs), wrow))
    return in_maps, metas, cnt, NQ


def aggregate(results, metas, cnt):
    total = 0.0
    for res, (npair, wrow) in zip(results, metas):
        v = np.asarray(res["outv"], np.float64).reshape(-1)
        negh = np.asarray(res["outh"], np.float64).reshape(-1)
        total += 0.5 * float(wrow.astype(np.float64) @ negh)
        total -= 0.5 * float(v[:npair].sum())
    total += cnt * np.log(2.0)
    return np.float32(total / cnt * 16.0)


def kernel(x, gt_s_labels, W, b, seen_att):
    x = np.asarray(x, np.float32)
    labels = np.asarray(gt_s_labels)
    W = np.asarray(W, np.float32)
    b = np.asarray(b, np.float32)
    seen_att = np.asarray(seen_att, np.float32)
    assert x.shape == (B, D) and W.shape == (ATT, D)
    assert seen_att.shape == (C, ATT) and labels.shape == (B,)
    in_maps, metas, cnt, NQ = prep_inputs(x, labels, W, b, seen_att)
    if cnt == 0:
        return np.float32(0.0)
    nc = _build_program(NQ)
    res = run_bass_kernel_spmd(nc, in_maps, core_ids=list(range(N_CORES)))
    return aggregate(res.results, metas, cnt)


if __name__ == "__main__":
    data = np.load("/root/problem/inputs_cache.npz")
    out = kernel(data["x"], data["gt_s_labels"], data["W"], data["b"],
                 data["seen_att"])
    print("kernel loss:", out)


# revision 30
# speedup vs baseline: 1.5264x; 1.2747x over previous
"""Trainium2 Bass kernel for nn_AdversarialLoss (pairwise JS loss over softmaxes).

Strategy (8 NeuronCores, no collectives):
  - Only pairs (i<j) with equal labels contribute. Pairs exist only inside label
    groups, so groups are assigned to cores (split if needed) and each core
    computes a partial sum over its own pairs using only its own rows of x.
  - Per core the device computes, for its (padded) row set:
        y   = x_rows @ W.T + b                      (bf16 matmul, f32 accum)
        G   = y @ seen_att.T                        (bf16 matmul)
        u   = 5 * G / (|y_i| * |sa_c|)              (= logits/TEMP)
        e   = exp(u - max), se = sum(e)             (softmax numerator)
        negh_i = sum_c p*logP = (sum e*u)/se - lse
        q_n = p_i + p_j  via fp32 matmul S'.T @ e   (S' = pair-select * 1/se)
        v_n = sum_c q*ln(q)
    and returns v [L] and negh [R]; the host combines
        loss = 16/cnt * ( sum_pairs(0.5*(negh_i+negh_j)) + cnt*ln2 - 0.5*sum v )
  - W.T / seen_att.T are read by every core (redundant, bf16 to halve bytes);
    x / pair-selection are sharded. Host does only O(B^2) index bookkeeping,
    layout swizzles, and the final dot with 0/1 pair-count weights.

Self-contained: hardcodes shapes from the problem spec (x[256,2048],
W[512,2048], b[512], seen_att[1024,512], labels[256]).
"""

import numpy as np
import ml_dtypes
from contextlib import ExitStack

import concourse.bacc as bacc
import concourse.tile as tile
import concourse.mybir as mybir
from concourse import masks
from concourse.bass_utils import run_bass_kernel_spmd
from concourse.hw_specs import get_activation_tables as _real_act_tables


def _act_tables_ln_exp_only(module_arch):
    """Keep only the one act-func set that covers ln+exp+square+copy so the
    table-load pass emits a single LoadActFuncSet instead of ping-ponging
    between per-function sets. Positions are preserved so set ids stay valid."""
    tables = _real_act_tables(module_arch)
    out = {}
    for name, funcs in tables.items():
        if name == "natural_log_exp_and_others":
            out[name] = funcs
        else:
            out[name] = set()
    return out


# NOTE: forcing every activation into act-func-set 6 ("natural_log_exp_and_
# others") costs ~10x accuracy on HW (rel err 2e-3 vs 2e-4) - its ln/exp
# tables are lower-precision than the per-function sets. Left disabled.

dt = mybir.dt
AF = mybir.ActivationFunctionType
ALU = mybir.AluOpType
AX = mybir.AxisListType

B, D, ATT, C = 256, 2048, 512, 1024
KD, KA = D // 128, ATT // 128   # K-chunks for the two matmuls
R = 128                         # padded rows per core (fixed -> one cached NEFF)
QCHUNK = 128                    # pairs per Q tile
N_CORES = 8

_BF = ml_dtypes.bfloat16
_F8 = ml_dtypes.float8_e4m3
M1_SCALE = 16.0  # pre-scale W/b so fp8 sees normal-range values; l2norm cancels it

_prog_cache: dict = {}


def _build_program(NQ: int):
    """Build the (input-independent) 8-core SPMD Bass program for NQ pair-tiles."""
    if NQ in _prog_cache:
        return _prog_cache[NQ]
    L = NQ * QCHUNK
    nc = bacc.Bacc("TRN2", target_bir_lowering=False, debug=False,
                   num_devices=N_CORES)

    xt_d = nc.dram_tensor("xt", [128, KD * R], dt.float8e4, kind="ExternalInput")
    wt_d = nc.dram_tensor("wt", [128, KD * ATT], dt.float8e4, kind="ExternalInput")
    sat_d = nc.dram_tensor("sat", [128, KA * C], dt.bfloat16, kind="ExternalInput")
    b_d = nc.dram_tensor("bias", [1, ATT], dt.float8e4, kind="ExternalInput")
    st_d = nc.dram_tensor("st", [R, L], dt.float8e4, kind="ExternalInput")
    if NQ == 1:
        # single [128, 2] output (col0 = v, col1 = negh): one tail DMA
        outall_d = nc.dram_tensor("outall", [R, 2], dt.float32,
                                  kind="ExternalOutput")
        outv_d = outh_d = None
    else:
        outall_d = None
        outv_d = nc.dram_tensor("outv", [L, 1], dt.float32, kind="ExternalOutput")
        outh_d = nc.dram_tensor("outh", [R, 1], dt.float32, kind="ExternalOutput")

    with tile.TileContext(nc) as tc, ExitStack() as ctx:
        io = ctx.enter_context(tc.tile_pool(name="io", bufs=1))
        wk = ctx.enter_context(tc.tile_pool(name="wk", bufs=1))
        ps = ctx.enter_context(tc.tile_pool(name="ps", bufs=1, space="PSUM"))

        # ---- input DMAs (HWDGE), ordered for earliest dependency release:
        # xt (M1 lhsT), sat halves (sa-norm chain), wt in 8 chunks pipelined
        # with M1, small tensors in between. ----
        xt_sb = io.tile([128, KD * R], dt.float8e4)
        nc.sync.dma_start(xt_sb[:], xt_d.ap())
        sat_sb = io.tile([128, KA * C], dt.bfloat16)
        for j in range(KA):  # chunked: each sasq_j starts as its chunk lands
            nc.sync.dma_start(sat_sb[:, j * C:(j + 1) * C],
                              sat_d.ap()[:, j * C:(j + 1) * C])
        st_sb = io.tile([R, L], dt.float8e4)
        nc.sync.dma_start(st_sb[:], st_d.ap())
        b_sb = io.tile([1, ATT], dt.float8e4)
        nc.sync.dma_start(b_sb[:], b_d.ap())
        wt_sb = io.tile([128, KD * ATT], dt.float8e4)
        wsl = [slice(w * 2 * ATT, (w + 1) * 2 * ATT) for w in range(8)]
        for w in range(8):
            nc.sync.dma_start(wt_sb[:, wsl[w]], wt_d.ap()[:, wsl[w]])

        # ---- constants ----
        ident = wk.tile([128, 128], dt.bfloat16)
        masks.make_identity(nc, ident[:])
        ones1R_f8 = wk.tile([1, R], dt.float8e4)
        nc.gpsimd.memset(ones1R_f8[:], 1.0)
        ones128_f = wk.tile([128, 1], dt.float32)
        nc.gpsimd.memset(ones128_f[:], 1.0)
        ones128_r = wk.tile([128, 1], dt.float32r)
        nc.vector.tensor_copy(ones128_r[:], ones128_f[:])
        st_r = wk.tile([R, L], dt.float32r)
        nc.vector.tensor_copy(st_r[:], st_sb[:])  # 0/1/2 values: exact in f32r

        # ---- seen_att column norms: nsq_c = sum_a sa[c,a]^2 (f32r matmuls) ----
        sasq = [wk.tile([128, C], dt.float32r, name=f"sasq{j}") for j in range(KA)]
        for j in range(KA):  # DVE keeps Square off ACT (fewer table loads)
            src = sat_sb[:, j * C:(j + 1) * C]
            nc.vector.tensor_tensor(sasq[j][:], src, src, ALU.mult)
        nsq_ps = ps.tile([1, C], dt.float32, tag="big", bufs=2)
        for j in range(KA):
            for h in range(2):
                nc.tensor.matmul(nsq_ps[:, h * 512:(h + 1) * 512],
                                 ones128_r[:], sasq[j][:, h * 512:(h + 1) * 512],
                                 start=(j == 0), stop=(j == KA - 1))
        # rn'_c = 1/max(|sa_c|, 1e-12) = exp(-0.5*ln(nsq + 1e-24))
        eps1 = wk.tile([1, 1], dt.float32)
        nc.gpsimd.memset(eps1[:], 1e-24)
        lnn = wk.tile([1, C], dt.float32)
        nc.scalar.activation(lnn[:], nsq_ps[:], AF.Ln, bias=eps1[:])
        # ---- M1: y = x @ W.T + b (fp8 DoubleRow: 256-wide K per pass) ----
        y_ps = ps.tile([R, ATT], dt.float32, tag="y")
        xt3 = xt_sb[:].rearrange("p (j ko r) -> p j ko r", ko=2, r=R)
        wt3 = wt_sb[:].rearrange("p (j ko a) -> p j ko a", ko=2, a=ATT)
        for k in range(KD // 2):
            nc.tensor.matmul(y_ps[:], xt3[:, k], wt3[:, k],
                             start=(k == 0), stop=False,
                             perf_mode=mybir.MatmulPerfMode.DoubleRow)
        nc.tensor.matmul(y_ps[:], ones1R_f8[:], b_sb[:], start=False, stop=True)

        # y -> bf16, transpose to yT for M2
        y_bf = wk.tile([R, ATT], dt.bfloat16)
        nc.vector.tensor_copy(y_bf[:], y_ps[:])
        yt_sb = wk.tile([128, KA * R], dt.bfloat16)
        for j in range(KA):
            yt_ps = ps.tile([128, R], dt.bfloat16, tag="t", bufs=1, name=f"ytp{j}")
            nc.tensor.transpose(yt_ps[:], y_bf[:, j * 128:(j + 1) * 128], ident[:])
            nc.vector.tensor_copy(yt_sb[:, j * R:(j + 1) * R], yt_ps[:])

        # row norms from bf16 y (stt-accum; keeps Square off ACT):
        # rn5_i = 5/max(|y_i|, 1e-12) = exp(-0.5*ln(max(ssq,1e-24)) + ln5)
        scr_y = wk.tile([R, ATT], dt.float32)
        rowssq = wk.tile([R, 1], dt.float32)
        nc.vector.scalar_tensor_tensor(scr_y[:], y_bf[:], 1.0, y_bf[:],
                                       op0=ALU.mult, op1=ALU.mult,
                                       accum_out=rowssq[:])
        from concourse.tile_rust import add_dep_helper as _adh
        epsR = wk.tile([R, 1], dt.float32)
        nc.gpsimd.memset(epsR[:], 1e-24)
        lnr = wk.tile([R, 1], dt.float32)
        nc.scalar.activation(lnr[:], rowssq[:], AF.Ln, bias=epsR[:])
        # Exp ops grouped after the two Ln ops: 3 act-table loads total
        rnp = wk.tile([1, C], dt.float32r)
        nc.scalar.activation(rnp[:], lnn[:], AF.Exp, scale=-0.5)
        ln5 = wk.tile([R, 1], dt.float32)
        nc.gpsimd.memset(ln5[:], float(np.log(5.0)))
        rn5 = wk.tile([R, 1], dt.float32)
        nc.scalar.activation(rn5[:], lnr[:], AF.Exp, scale=-0.5, bias=ln5[:])

        # ---- M2: G = y @ saT ----
        g_ps = ps.tile([R, C], dt.float32, tag="big", bufs=2)
        for j in range(KA):
            for h in range(2):
                nc.tensor.matmul(g_ps[:, h * 512:(h + 1) * 512],
                                 yt_sb[:, j * R:(j + 1) * R],
                                 sat_sb[:, j * C + h * 512: j * C + (h + 1) * 512],
                                 start=(j == 0), stop=(j == KA - 1))

        # broadcast rn' to all partitions on the (otherwise idle) GpSimd engine
        RN = wk.tile([R, C], dt.float32r)
        nc.gpsimd.partition_broadcast(RN[:], rnp[:])

        # ---- u_raw = G * rn'_c ; softmax e = exp(rn5_i * u_raw) (|u| <= 5:
        # no max needed). rn5 enters as ACT Exp's per-partition scale so the
        # u computation never waits on the row-norm chain. Split into C-halves
        # so ACT/DVE/PE pipeline. ----
        u = wk.tile([R, C], dt.float32)
        seh = [wk.tile([R, 1], dt.float32, name=f"seh{h}") for h in range(2)]
        e = wk.tile([R, C], dt.float32)
        for h in range(2):
            sl = slice(h * 512, (h + 1) * 512)
            nc.vector.tensor_tensor(u[:, sl], g_ps[:, sl], RN[:, sl], ALU.mult)
            nc.scalar.activation(e[:, sl], u[:, sl], AF.Exp, scale=rn5[:],
                                 accum_out=seh[h][:])
        se = wk.tile([R, 1], dt.float32)
        nc.vector.tensor_tensor(se[:], seh[0][:], seh[1][:], ALU.add)
        rse = wk.tile([R, 1], dt.float32)
        nc.vector.reciprocal(rse[:], se[:])
        p_r = wk.tile([R, C], dt.float32r)
        p_r_inst = None
        for h in range(2):
            sl = slice(h * 512, (h + 1) * 512)
            p_r_inst = nc.vector.tensor_scalar_mul(p_r[:, sl], e[:, sl], rse[:])

        # ---- pairs: q = S.T @ P (f32r), v = sum_c q*ln(q) ----
        comb = wk.tile([R, 2], dt.float32, name="comb") if NQ == 1 else None
        for qi in range(NQ):
            if NQ == 1:
                v = comb[:, 0:1]
            else:
                v = wk.tile([QCHUNK, 1], dt.float32, tag="v", bufs=2,
                            name=f"v{qi}")
            vh = [wk.tile([QCHUNK, 1], dt.float32, tag=f"vh{h}", bufs=2,
                          name=f"vh{qi}_{h}") for h in range(2)]
            for h in range(2):
                q_ps = ps.tile([QCHUNK, 512], dt.float32, tag=f"qh{h}", bufs=1,
                               name=f"qps{qi}_{h}")
                nc.tensor.matmul(q_ps[:],
                                 st_r[:, qi * QCHUNK:(qi + 1) * QCHUNK],
                                 p_r[:, h * 512:(h + 1) * 512],
                                 start=True, stop=True)
                lnq = wk.tile([QCHUNK, 512], dt.float32, tag="lnq", bufs=2,
                              name=f"lnq{qi}_{h}")
                scr3 = wk.tile([QCHUNK, 512], dt.float32, tag="scr3", bufs=2,
                               name=f"scr3{qi}_{h}")
                nc.scalar.activation(lnq[:], q_ps[:], AF.Ln)
                nc.vector.scalar_tensor_tensor(
                    scr3[:], q_ps[:], 1.0, lnq[:],
                    op0=ALU.mult, op1=ALU.mult, accum_out=vh[h][:])
            nc.vector.tensor_tensor(v[:], vh[0][:], vh[1][:], ALU.add)
            if NQ != 1:
                nc.sync.dma_start(outv_d.ap()[qi * QCHUNK:(qi + 1) * QCHUNK, :],
                                  v[:])

        # ---- negh = (sum_c e*u)/se - ln(se)  (emitted last: fills gaps) ----
        scr2 = wk.tile([R, C], dt.float32)
        t1h = [wk.tile([R, 1], dt.float32, name=f"t1h{h}") for h in range(2)]
        for h in range(2):
            sl = slice(h * 512, (h + 1) * 512)
            t1_inst = nc.vector.scalar_tensor_tensor(scr2[:, sl], e[:, sl], 1.0,
                                                     u[:, sl], op0=ALU.mult,
                                                     op1=ALU.mult,
                                                     accum_out=t1h[h][:])
            _adh(t1_inst.ins, p_r_inst.ins,
                 reason="keep negh accumulation off the pair critical path")
        t1r = wk.tile([R, 1], dt.float32)
        nc.vector.tensor_tensor(t1r[:], t1h[0][:], t1h[1][:], ALU.add)
        t1 = wk.tile([R, 1], dt.float32)
        nc.vector.tensor_tensor(t1[:], t1r[:], rn5[:], ALU.mult)
        lnse = wk.tile([R, 1], dt.float32)
        nc.scalar.activation(lnse[:], se[:], AF.Ln)
        if NQ == 1:
            negh = comb[:, 1:2]
        else:
            negh = wk.tile([R, 1], dt.float32, name="negh")
        nc.vector.scalar_tensor_tensor(negh[:], t1[:], rse[:], lnse[:],
                                       op0=ALU.mult, op1=ALU.subtract)
        if NQ == 1:
            nc.sync.dma_start(outall_d.ap(), comb[:])
        else:
            nc.sync.dma_start(outh_d.ap(), negh[:])

    nc.compile()
    _prog_cache[NQ] = nc
    return nc


def _shard_pairs(labels):
    groups: dict = {}
    for i, g in enumerate(labels.tolist()):
        groups.setdefault(g, []).append(i)
    group_pairs = []
    for rows in groups.values():
        ps = [(rows[a], rows[b])
              for a in range(len(rows)) for b in range(a + 1, len(rows))]
        if ps:
            group_pairs.append(ps)
    cnt = sum(len(p) for p in group_pairs)
    if cnt == 0:
        return None, 0
    group_pairs.sort(key=len, reverse=True)
    core_pairs = [[] for _ in range(N_CORES)]
    cap = max(1, (cnt + N_CORES - 1) // N_CORES)
    for ps in group_pairs:
        k = min(range(N_CORES), key=lambda kk: len(core_pairs[kk]))
        while len(ps) > cap:
            core_pairs[k].extend(ps[:cap])
            ps = ps[cap:]
            k = min(range(N_CORES), key=lambda kk: len(core_pairs[kk]))
        core_pairs[k].extend(ps)
    return core_pairs, cnt


def _swizzle_kmaj(a2d, kchunks):
    """[Ktot, N] -> [128, kchunks*N] with element (p, k*N+n) = a[k*128+p, n]."""
    ktot, n = a2d.shape
    assert ktot == kchunks * 128
    return np.ascontiguousarray(
        a2d.reshape(kchunks, 128, n).transpose(1, 0, 2).reshape(128, kchunks * n))


def _swizzle_dr(a2d):
    """[Ktot, N] -> [128, (Ktot//256)*2*N] DoubleRow layout:
    element (p, ((j*2+ko)*N+n)) = a[j*256 + ko*128 + p, n]."""
    ktot, n = a2d.shape
    assert ktot % 256 == 0
    j = ktot // 256
    return np.ascontiguousarray(
        a2d.reshape(j, 2, 128, n).transpose(2, 0, 1, 3).reshape(128, j * 2 * n))


def prep_inputs(x, labels, W, b, seen_att):
    """Host-side sharding/layout. Returns (in_maps, per_core_meta, cnt, NQ)."""
    core_pairs, cnt = _shard_pairs(labels)
    if cnt == 0:
        return None, None, 0, 0
    NQ = (max(len(p) for p in core_pairs) + QCHUNK - 1) // QCHUNK
    L = NQ * QCHUNK
    wt = (_swizzle_dr(np.ascontiguousarray(W.T)) * M1_SCALE).astype(_F8)
    sat = _swizzle_kmaj(np.ascontiguousarray(seen_att.T), KA).astype(_BF)
    b_row = (np.asarray(b, np.float32).reshape(1, ATT) * M1_SCALE).astype(_F8)
    in_maps, metas = [], []
    for k in range(N_CORES):
        pairs = core_pairs[k]
        rows = sorted({r for p in pairs for r in p})
        assert len(rows) <= R, f"core {k}: row set {len(rows)} exceeds {R}"
        ridx = {r: a for a, r in enumerate(rows)}
        xk = np.zeros((D, R), np.float32)
        if rows:
            xk[:, :len(rows)] = np.asarray(x, np.float32)[rows].T
        st = np.zeros((R, L), np.float32)
        for n, (i, j) in enumerate(pairs):
            st[ridx[i], n] = 1.0
            st[ridx[j], n] = 1.0
        for n in range(len(pairs), L):
            st[0, n] = 2.0  # benign padding: q = 2*p_row0 > 0
        wrow = np.zeros(R, np.float32)
        for (i, j) in pairs:
            wrow[ridx[i]] += 1.0
            wrow[ridx[j]] += 1.0
        in_maps.append({
            "xt": _swizzle_dr(xk).astype(_F8),
            "wt": wt, "sat": sat, "bias": b_row,
            "st": st.astype(_F8),
        })
        metas.append((len(pairs), wrow))
    return in_maps, metas, cnt, NQ


def aggregate(results, metas, cnt):
    total = 0.0
    for res, (npair, wrow) in zip(results, metas):
        if "outall" in res:
            both = np.asarray(res["outall"], np.float64)
            v, negh = both[:, 0], both[:, 1]
        else:
            v = np.asarray(res["outv"], np.float64).reshape(-1)
            negh = np.asarray(res["outh"], np.float64).reshape(-1)
        total += 0.5 * float(wrow.astype(np.float64) @ negh)
        total -= 0.5 * float(v[:npair].sum())
    total += cnt * np.log(2.0)
    return np.float32(total / cnt * 16.0)


def kernel(x, gt_s_labels, W, b, seen_att):
    x = np.asarray(x, np.float32)
    labels = np.asarray(gt_s_labels)
    W = np.asarray(W, np.float32)
    b = np.asarray(b, np.float32)
    seen_att = np.asarray(seen_att, np.float32)
    assert x.shape == (B, D) and W.shape == (ATT, D)
    assert seen_att.shape == (C, ATT) and labels.shape == (B,)
    in_maps, metas, cnt, NQ = prep_inputs(x, labels, W, b, seen_att)
    if cnt == 0:
        return np.float32(0.0)
    nc = _build_program(NQ)
    res = run_bass_kernel_spmd(nc, in_maps, core_ids=list(range(N_CORES)))
    return aggregate(res.results, metas, cnt)


if __name__ == "__main__":
    data = np.load("/root/problem/inputs_cache.npz")
    out = kernel(data["x"], data["gt_s_labels"], data["W"], data["b"],
                 data["seen_att"])
    print("kernel loss:", out)


# revision 42
# speedup vs baseline: 1.5722x; 1.0300x over previous
"""Trainium2 Bass kernel for nn_AdversarialLoss (pairwise JS loss over softmaxes).

Strategy (8 NeuronCores, no collectives):
  - Only pairs (i<j) with equal labels contribute. Pairs exist only inside label
    groups, so groups are assigned to cores (split if needed) and each core
    computes a partial sum over its own pairs using only its own rows of x.
  - Per core the device computes, for its (padded) row set:
        y   = x_rows @ W.T + b          (fp8 DoubleRow matmul, f32 accum;
                                         W,b host-prescaled x16 - the row
                                         l2norm cancels any scale)
        G   = y @ seen_att.T            (bf16 matmul)
        u   = G * rn'_c                 (rn' = 1/|sa_c| via ln/exp on ACT)
        e   = exp(rn5_i * u), se = sum(e)   (rn5 = 5/|y_i| as ACT Exp scale;
                                         |logits/TEMP| <= 5 so no max needed)
        negh_i = sum_c p*logP = rn5*(sum e*u)/se - ln(se)
        q_n = p_i + p_j  via f32r matmul S.T @ P (S exact 0/1/2, P = e/se)
        v_n = sum_c q*ln(q)
    and returns v [L] and negh [R]; the host combines
        loss = 16/cnt * ( sum_pairs(0.5*(negh_i+negh_j)) + cnt*ln2 - 0.5*sum v )
  - W.T / seen_att.T are read by every core (redundant, fp8/bf16 to shrink
    the DMA floor); x / pair-selection are sharded. Host does only O(B^2)
    index bookkeeping, layout swizzles, and the final 0/1-weighted dots.

Self-contained: hardcodes shapes from the problem spec (x[256,2048],
W[512,2048], b[512], seen_att[1024,512], labels[256]).
"""

import numpy as np
import ml_dtypes
from contextlib import ExitStack

import concourse.bacc as bacc
import concourse.tile as tile
import concourse.mybir as mybir
from concourse import masks
from concourse.bass_utils import run_bass_kernel_spmd
from concourse.hw_specs import get_activation_tables as _real_act_tables


def _act_tables_ln_exp_only(module_arch):
    """Keep only the one act-func set that covers ln+exp+square+copy so the
    table-load pass emits a single LoadActFuncSet instead of ping-ponging
    between per-function sets. Positions are preserved so set ids stay valid."""
    tables = _real_act_tables(module_arch)
    out = {}
    for name, funcs in tables.items():
        if name == "natural_log_exp_and_others":
            out[name] = funcs
        else:
            out[name] = set()
    return out


# NOTE: forcing every activation into act-func-set 6 ("natural_log_exp_and_
# others") costs ~10x accuracy on HW (rel err 2e-3 vs 2e-4) - its ln/exp
# tables are lower-precision than the per-function sets. Left disabled.

dt = mybir.dt
AF = mybir.ActivationFunctionType
ALU = mybir.AluOpType
AX = mybir.AxisListType

B, D, ATT, C = 256, 2048, 512, 1024
KD, KA = D // 128, ATT // 128   # K-chunks for the two matmuls
R = 128                         # padded rows per core (fixed -> one cached NEFF)
QCHUNK = 128                    # pairs per Q tile
N_CORES = 8

_BF = ml_dtypes.bfloat16
_F8 = ml_dtypes.float8_e4m3
M1_SCALE = 16.0  # pre-scale W/b so fp8 sees normal-range values; l2norm cancels it

_prog_cache: dict = {}


def _build_program(NQ: int):
    """Build the (input-independent) 8-core SPMD Bass program for NQ pair-tiles."""
    if NQ in _prog_cache:
        return _prog_cache[NQ]
    L = NQ * QCHUNK
    nc = bacc.Bacc("TRN2", target_bir_lowering=False, debug=False,
                   num_devices=N_CORES)

    PKW = KD * R + KD * ATT + L   # packed fp8 input: [ xt | wt | st ]
    pk_d = nc.dram_tensor("pk", [128, PKW], dt.float8e4, kind="ExternalInput")
    sat_d = nc.dram_tensor("sat", [128, KA * C], dt.bfloat16, kind="ExternalInput")
    b_d = nc.dram_tensor("bias", [1, ATT], dt.float8e4, kind="ExternalInput")
    if NQ == 1:
        # single [128, 2] output (col0 = v, col1 = negh): one tail DMA
        outall_d = nc.dram_tensor("outall", [R, 2], dt.float32,
                                  kind="ExternalOutput")
        outv_d = outh_d = None
    else:
        outall_d = None
        outv_d = nc.dram_tensor("outv", [L, 1], dt.float32, kind="ExternalOutput")
        outh_d = nc.dram_tensor("outh", [R, 1], dt.float32, kind="ExternalOutput")

    with tile.TileContext(nc) as tc, ExitStack() as ctx:
        io = ctx.enter_context(tc.tile_pool(name="io", bufs=1))
        wk = ctx.enter_context(tc.tile_pool(name="wk", bufs=1))
        ps = ctx.enter_context(tc.tile_pool(name="ps", bufs=1, space="PSUM"))

        # ---- input DMAs (HWDGE), ordered for earliest dependency release:
        # xt (M1 lhsT), sat halves (sa-norm chain), wt in 8 chunks pipelined
        # with M1, small tensors in between. ----
        # HWDGE issue costs ~0.6us per dma_start, so inputs arrive as SIX
        # DMAs: b, then sat halves interleaved with three chunks of the
        # packed fp8 tensor. Chunk boundaries preserve the release order
        # (sat half -> sasq work; packed chunks -> M1 K-chunk pacing).
        b_sb = io.tile([1, ATT], dt.float8e4)
        nc.sync.dma_start(b_sb[:], b_d.ap())
        sat_sb = io.tile([128, KA * C], dt.bfloat16)
        pk_sb = io.tile([128, PKW], dt.float8e4)
        XT0, WT0, ST0 = 0, KD * R, KD * R + KD * ATT
        cuts = [0, WT0 + 2 * ATT, WT0 + 6 * ATT, PKW]
        nc.sync.dma_start(sat_sb[:, :2 * C], sat_d.ap()[:, :2 * C])
        nc.sync.dma_start(pk_sb[:, cuts[0]:cuts[1]], pk_d.ap()[:, cuts[0]:cuts[1]])
        nc.sync.dma_start(sat_sb[:, 2 * C:], sat_d.ap()[:, 2 * C:])
        nc.sync.dma_start(pk_sb[:, cuts[1]:cuts[2]], pk_d.ap()[:, cuts[1]:cuts[2]])
        nc.sync.dma_start(pk_sb[:, cuts[2]:cuts[3]], pk_d.ap()[:, cuts[2]:cuts[3]])
        xt_sb = pk_sb[:, XT0:XT0 + KD * R]
        wt_sb = pk_sb[:, WT0:WT0 + KD * ATT]
        st_sb = pk_sb[:, ST0:ST0 + L]

        # ---- constants ----
        ident = wk.tile([128, 128], dt.bfloat16)
        masks.make_identity(nc, ident[:])
        dum = wk.tile([1, 1], dt.float32)
        nc.gpsimd.memset(dum[:], 1.0)
        dum2 = wk.tile([1, 1], dt.float32)
        nc.scalar.activation(dum2[:], dum[:], AF.Ln)  # pins Ln table load early
        ones1R_f8 = wk.tile([1, R], dt.float8e4)
        nc.gpsimd.memset(ones1R_f8[:], 1.0)
        ones128_f = wk.tile([128, 1], dt.float32)
        nc.gpsimd.memset(ones128_f[:], 1.0)
        ones128_r = wk.tile([128, 1], dt.float32r)
        nc.vector.tensor_copy(ones128_r[:], ones128_f[:])
        st_r = wk.tile([R, L], dt.float32r)
        nc.vector.tensor_copy(st_r[:], st_sb)  # 0/1/2 values: exact in f32r

        # ---- M1: y = x @ W.T + b (fp8 DoubleRow: 256-wide K per pass) ----
        y_ps = ps.tile([R, ATT], dt.float32, tag="y")
        xt3 = xt_sb.rearrange("p (j ko r) -> p j ko r", ko=2, r=R)
        wt3 = wt_sb.rearrange("p (j ko a) -> p j ko a", ko=2, a=ATT)
        for k in range(KD // 2):
            nc.tensor.matmul(y_ps[:], xt3[:, k], wt3[:, k],
                             start=(k == 0), stop=False,
                             perf_mode=mybir.MatmulPerfMode.DoubleRow)
        nc.tensor.matmul(y_ps[:], ones1R_f8[:], b_sb[:], start=False, stop=True)

        # ---- seen_att column norms: nsq_c = sum_a sa[c,a]^2 (f32r matmuls) ----
        sasq = [wk.tile([128, C], dt.float32r, name=f"sasq{j}") for j in range(KA)]
        for j in range(KA):  # split DVE/ACT so the squares aren't serial
            src = sat_sb[:, j * C:(j + 1) * C]
            if j % 2 == 0:
                nc.vector.tensor_tensor(sasq[j][:], src, src, ALU.mult)
            else:
                nc.scalar.activation(sasq[j][:], src, AF.Square)
        nsq_ps = ps.tile([1, C], dt.float32, tag="big", bufs=2)
        for j in range(KA):
            for h in range(2):
                nc.tensor.matmul(nsq_ps[:, h * 512:(h + 1) * 512],
                                 ones128_r[:], sasq[j][:, h * 512:(h + 1) * 512],
                                 start=(j == 0), stop=(j == KA - 1))
        # rn'_c = 1/max(|sa_c|, 1e-12) = exp(-0.5*ln(nsq + 1e-24))
        eps1 = wk.tile([1, 1], dt.float32)
        nc.gpsimd.memset(eps1[:], 1e-24)
        lnn = wk.tile([1, C], dt.float32)
        rnp = wk.tile([1, C], dt.float32r)
        RN = wk.tile([R, C], dt.float32r)
        for h in range(2):  # C-halves: ln -> exp -> broadcast pipeline
            sl = slice(h * 512, (h + 1) * 512)
            nc.scalar.activation(lnn[:, sl], nsq_ps[:, sl], AF.Ln, bias=eps1[:])
            nc.scalar.activation(rnp[:, sl], lnn[:, sl], AF.Exp, scale=-0.5)
            nc.gpsimd.partition_broadcast(RN[:, sl], rnp[:, sl])
        # y -> bf16, transpose to yT for M2
        y_bf = wk.tile([R, ATT], dt.bfloat16)
        nc.vector.tensor_copy(y_bf[:], y_ps[:])
        yt_sb = wk.tile([128, KA * R], dt.bfloat16)
        for j in range(KA):
            yt_ps = ps.tile([128, R], dt.bfloat16, tag="t", bufs=1, name=f"ytp{j}")
            nc.tensor.transpose(yt_ps[:], y_bf[:, j * 128:(j + 1) * 128], ident[:])
            nc.vector.tensor_copy(yt_sb[:, j * R:(j + 1) * R], yt_ps[:])

        # ---- M2: G = y @ saT ----
        g_ps = ps.tile([R, C], dt.float32, tag="big", bufs=2)
        for j in range(KA):
            for h in range(2):
                nc.tensor.matmul(g_ps[:, h * 512:(h + 1) * 512],
                                 yt_sb[:, j * R:(j + 1) * R],
                                 sat_sb[:, j * C + h * 512: j * C + (h + 1) * 512],
                                 start=(j == 0), stop=(j == KA - 1))

        # row norms from bf16 y (stt-accum; keeps Square off ACT):
        # rn5_i = 5/max(|y_i|, 1e-12) = exp(-0.5*ln(max(ssq,1e-24)) + ln5)
        scr_y = wk.tile([R, ATT], dt.float32)
        rowssq = wk.tile([R, 1], dt.float32)
        nc.vector.scalar_tensor_tensor(scr_y[:], y_bf[:], 1.0, y_bf[:],
                                       op0=ALU.mult, op1=ALU.mult,
                                       accum_out=rowssq[:])
        from concourse.tile_rust import add_dep_helper as _adh
        epsR = wk.tile([R, 1], dt.float32)
        nc.gpsimd.memset(epsR[:], 1e-24)
        lnr = wk.tile([R, 1], dt.float32)
        nc.scalar.activation(lnr[:], rowssq[:], AF.Ln, bias=epsR[:])
        ln5 = wk.tile([R, 1], dt.float32)
        nc.gpsimd.memset(ln5[:], float(np.log(5.0)))
        rn5 = wk.tile([R, 1], dt.float32)
        nc.scalar.activation(rn5[:], lnr[:], AF.Exp, scale=-0.5, bias=ln5[:])

        # ---- u_raw = G * rn'_c ; softmax e = exp(rn5_i * u_raw) (|u| <= 5:
        # no max needed). rn5 enters as ACT Exp's per-partition scale so the
        # u computation never waits on the row-norm chain. Split into C-halves
        # so ACT/DVE/PE pipeline. ----
        u = wk.tile([R, C], dt.float32)
        seh = [wk.tile([R, 1], dt.float32, name=f"seh{h}") for h in range(2)]
        e = wk.tile([R, C], dt.float32)
        for h in range(2):
            sl = slice(h * 512, (h + 1) * 512)
            nc.vector.tensor_tensor(u[:, sl], g_ps[:, sl], RN[:, sl], ALU.mult)
            nc.scalar.activation(e[:, sl], u[:, sl], AF.Exp, scale=rn5[:],
                                 accum_out=seh[h][:])
        se = wk.tile([R, 1], dt.float32)
        nc.vector.tensor_tensor(se[:], seh[0][:], seh[1][:], ALU.add)
        rse = wk.tile([R, 1], dt.float32)
        nc.vector.reciprocal(rse[:], se[:])
        p_r = wk.tile([R, C], dt.float32r)
        p_r_inst = None
        for h in range(2):
            sl = slice(h * 512, (h + 1) * 512)
            p_r_inst = nc.vector.tensor_scalar_mul(p_r[:, sl], e[:, sl], rse[:])

        # ---- pairs: q = S.T @ P (f32r), v = sum_c q*ln(q) ----
        comb = wk.tile([R, 2], dt.float32, name="comb") if NQ == 1 else None
        for qi in range(NQ):
            if NQ == 1:
                v = comb[:, 0:1]
            else:
                v = wk.tile([QCHUNK, 1], dt.float32, tag="v", bufs=2,
                            name=f"v{qi}")
            vh = [wk.tile([QCHUNK, 1], dt.float32, tag=f"vh{h}", bufs=2,
                          name=f"vh{qi}_{h}") for h in range(2)]
            for h in range(2):
                q_ps = ps.tile([QCHUNK, 512], dt.float32, tag=f"qh{h}", bufs=1,
                               name=f"qps{qi}_{h}")
                nc.tensor.matmul(q_ps[:],
                                 st_r[:, qi * QCHUNK:(qi + 1) * QCHUNK],
                                 p_r[:, h * 512:(h + 1) * 512],
                                 start=True, stop=True)
                lnq = wk.tile([QCHUNK, 512], dt.float32, tag="lnq", bufs=2,
                              name=f"lnq{qi}_{h}")
                scr3 = wk.tile([QCHUNK, 512], dt.float32, tag="scr3", bufs=2,
                               name=f"scr3{qi}_{h}")
                nc.scalar.activation(lnq[:], q_ps[:], AF.Ln)
                nc.vector.scalar_tensor_tensor(
                    scr3[:], q_ps[:], 1.0, lnq[:],
                    op0=ALU.mult, op1=ALU.mult, accum_out=vh[h][:])
            nc.vector.tensor_tensor(v[:], vh[0][:], vh[1][:], ALU.add)
            if NQ != 1:
                nc.sync.dma_start(outv_d.ap()[qi * QCHUNK:(qi + 1) * QCHUNK, :],
                                  v[:])

        # ---- negh = (sum_c e*u)/se - ln(se)  (emitted last: fills gaps) ----
        scr2 = wk.tile([R, C], dt.float32)
        t1h = [wk.tile([R, 1], dt.float32, name=f"t1h{h}") for h in range(2)]
        for h in range(2):
            sl = slice(h * 512, (h + 1) * 512)
            t1_inst = nc.vector.scalar_tensor_tensor(scr2[:, sl], e[:, sl], 1.0,
                                                     u[:, sl], op0=ALU.mult,
                                                     op1=ALU.mult,
                                                     accum_out=t1h[h][:])
            _adh(t1_inst.ins, p_r_inst.ins,
                 reason="keep negh accumulation off the pair critical path")
        t1r = wk.tile([R, 1], dt.float32)
        nc.vector.tensor_tensor(t1r[:], t1h[0][:], t1h[1][:], ALU.add)
        t1 = wk.tile([R, 1], dt.float32)
        nc.vector.tensor_tensor(t1[:], t1r[:], rn5[:], ALU.mult)
        lnse = wk.tile([R, 1], dt.float32)
        nc.scalar.activation(lnse[:], se[:], AF.Ln)
        if NQ == 1:
            negh = comb[:, 1:2]
        else:
            negh = wk.tile([R, 1], dt.float32, name="negh")
        nc.vector.scalar_tensor_tensor(negh[:], t1[:], rse[:], lnse[:],
                                       op0=ALU.mult, op1=ALU.subtract)
        if NQ == 1:
            nc.sync.dma_start(outall_d.ap(), comb[:])
        else:
            nc.sync.dma_start(outh_d.ap(), negh[:])

    nc.compile()
    _prog_cache[NQ] = nc
    return nc


def _shard_pairs(labels):
    groups: dict = {}
    for i, g in enumerate(labels.tolist()):
        groups.setdefault(g, []).append(i)
    group_pairs = []
    for rows in groups.values():
        ps = [(rows[a], rows[b])
              for a in range(len(rows)) for b in range(a + 1, len(rows))]
        if ps:
            group_pairs.append(ps)
    cnt = sum(len(p) for p in group_pairs)
    if cnt == 0:
        return None, 0
    group_pairs.sort(key=len, reverse=True)
    core_pairs = [[] for _ in range(N_CORES)]
    cap = max(1, (cnt + N_CORES - 1) // N_CORES)
    for ps in group_pairs:
        k = min(range(N_CORES), key=lambda kk: len(core_pairs[kk]))
        while len(ps) > cap:
            core_pairs[k].extend(ps[:cap])
            ps = ps[cap:]
            k = min(range(N_CORES), key=lambda kk: len(core_pairs[kk]))
        core_pairs[k].extend(ps)
    return core_pairs, cnt


def _swizzle_kmaj(a2d, kchunks):
    """[Ktot, N] -> [128, kchunks*N] with element (p, k*N+n) = a[k*128+p, n]."""
    ktot, n = a2d.shape
    assert ktot == kchunks * 128
    return np.ascontiguousarray(
        a2d.reshape(kchunks, 128, n).transpose(1, 0, 2).reshape(128, kchunks * n))


def _swizzle_dr(a2d):
    """[Ktot, N] -> [128, (Ktot//256)*2*N] DoubleRow layout:
    element (p, ((j*2+ko)*N+n)) = a[j*256 + ko*128 + p, n]."""
    ktot, n = a2d.shape
    assert ktot % 256 == 0
    j = ktot // 256
    return np.ascontiguousarray(
        a2d.reshape(j, 2, 128, n).transpose(2, 0, 1, 3).reshape(128, j * 2 * n))


def prep_inputs(x, labels, W, b, seen_att):
    """Host-side sharding/layout. Returns (in_maps, per_core_meta, cnt, NQ)."""
    core_pairs, cnt = _shard_pairs(labels)
    if cnt == 0:
        return None, None, 0, 0
    NQ = (max(len(p) for p in core_pairs) + QCHUNK - 1) // QCHUNK
    L = NQ * QCHUNK
    wt = (_swizzle_dr(np.ascontiguousarray(W.T)) * M1_SCALE).astype(_F8)
    sat = _swizzle_kmaj(np.ascontiguousarray(seen_att.T), KA).astype(_BF)
    b_row = (np.asarray(b, np.float32).reshape(1, ATT) * M1_SCALE).astype(_F8)
    in_maps, metas = [], []
    for k in range(N_CORES):
        pairs = core_pairs[k]
        rows = sorted({r for p in pairs for r in p})
        assert len(rows) <= R, f"core {k}: row set {len(rows)} exceeds {R}"
        ridx = {r: a for a, r in enumerate(rows)}
        xk = np.zeros((D, R), np.float32)
        if rows:
            xk[:, :len(rows)] = np.asarray(x, np.float32)[rows].T
        st = np.zeros((R, L), np.float32)
        for n, (i, j) in enumerate(pairs):
            st[ridx[i], n] = 1.0
            st[ridx[j], n] = 1.0
        for n in range(len(pairs), L):
            st[0, n] = 2.0  # benign padding: q = 2*p_row0 > 0
        wrow = np.zeros(R, np.float32)
        for (i, j) in pairs:
            wrow[ridx[i]] += 1.0
            wrow[ridx[j]] += 1.0
        in_maps.append({
            "pk": np.concatenate(
                [_swizzle_dr(xk).astype(_F8), wt, st.astype(_F8)], axis=1),
            "sat": sat, "bias": b_row,
        })
        metas.append((len(pairs), wrow))
    return in_maps, metas, cnt, NQ


def aggregate(results, metas, cnt):
    total = 0.0
    for res, (npair, wrow) in zip(results, metas):
        if "outall" in res:
            both = np.asarray(res["outall"], np.float64)
            v, negh = both[:, 0], both[:, 1]
        else:
            v = np.asarray(res["outv"], np.float64).reshape(-1)
            negh = np.asarray(res["outh"], np.float64).reshape(-1)
        total += 0.5 * float(wrow.astype(np.float64) @ negh)
        total -= 0.5 * float(v[:npair].sum())
    total += cnt * np.log(2.0)
    return np.float32(total / cnt * 16.0)


def kernel(x, gt_s_labels, W, b, seen_att):
    x = np.asarray(x, np.float32)
    labels = np.asarray(gt_s_labels)
    W = np.asarray(W, np.float32)
    b = np.asarray(b, np.float32)
    seen_att = np.asarray(seen_att, np.float32)
    assert x.shape == (B, D) and W.shape == (ATT, D)
    assert seen_att.shape == (C, ATT) and labels.shape == (B,)
    in_maps, metas, cnt, NQ = prep_inputs(x, labels, W, b, seen_att)
    if cnt == 0:
        return np.float32(0.0)
    nc = _build_program(NQ)
    res = run_bass_kernel_spmd(nc, in_maps, core_ids=list(range(N_CORES)))
    return aggregate(res.results, metas, cnt)


if __name__ == "__main__":
    data = np.load("/root/problem/inputs_cache.npz")
    out = kernel(data["x"], data["gt_s_labels"], data["W"], data["b"],
                 data["seen_att"])
    print("kernel loss:", out)
